# revision 7
# baseline (speedup 1.0000x reference)
"""DIEN AUGRU kernel for 8 Trainium2 NeuronCores (Bass/Tile, SPMD data-parallel).

Strategy
--------
Batch 1024 is sharded 8x128 across cores (weights replicated); the recurrence
is serial in time only, so no cross-core communication is needed.

Per core (b=128 batch rows, S=200 steps, D=256):

* Attention phase (fp32): v[b,d] = sum_e Wa[d,e] item[b,e] via PE; then
  scores[b,t] = sum_d x[b,t,d] v[b,d] with one fused DVE tensor_tensor_reduce
  per step; softmax along t on ACT/DVE; the attention row for each step is
  transposed and parked in DRAM so the recurrence can partition-broadcast it.

* Recurrence phase (fp16 matmuls, fp32 PSUM accumulation): everything lives in
  an [e-on-partitions, b-on-free] layout, so the gate pre-activations come out
  of the PE directly and the new hidden state is produced by the elementwise
  chain already transposed for the next step's matmul (no transpose on the
  critical path).  For each gate channel c and 128-wide e-chunk j:
      P_c[:, j] = bias_c + x_t @ W_c + h @ U_c      (PSUM accumulation)
  with the bias injected by one K=2 matmul (bias rows x a 0/1 selector), the
  x-side matmuls issued one step ahead of the h-side matmuls so the PE stays
  busy while the elementwise chain for the previous step runs.

Chain per step: r = sigmoid(P_r); u = sigmoid(P_u); hh = tanh(r*P_h2 + P_h1);
uh = attn_t * u; h' = h*(1-uh) + hh*uh.  ACT does the sigmoids/tanh straight
from PSUM, DVE does the tensor-tensor arithmetic, GPSIMD does the attention
gate pieces off the critical path.

Outputs are written as fp16 ([b,t,d] via DMA-transpose of the fp16 hidden
state) and upcast to fp32 on the host after the gather.
"""

import os
import sys
import json

sys.path.insert(0, "/opt/trn_rl_repo")

import numpy as np

import concourse.bass as bass
import concourse.tile as tile
from concourse import mybir
from concourse.bass import ts
from concourse.masks import make_identity
from contextlib import ExitStack

# ---------------------------------------------------------------------------
# Workarounds for this container's walrus build, which rejects any instruction
# carrying more than one sync-wait ("Too many sync wait commands").  Extra
# waits are hoisted onto same-engine NoOps inserted just before the offender,
# and the Tile tail drain is emitted as a chain of single-wait drains.
# ---------------------------------------------------------------------------
from bass_rust import ScopedClock

_MAX_WAITS = 1
_fix_counter = [0]


def _fix_instructions(insts):
    out = []
    for inst in insts:
        si = inst.get("sync_info")
        waits = (si or {}).get("on_wait") or []
        if len(waits) > _MAX_WAITS:
            keep = waits[-_MAX_WAITS:]
            extra = waits[:-_MAX_WAITS]
            for i in range(0, len(extra), _MAX_WAITS):
                _fix_counter[0] += 1
                out.append(
                    {
                        "debug": inst.get("debug", 0),
                        "engine": inst["engine"],
                        "ins": [],
                        "name": f"I-waitfix-{_fix_counter[0]}",
                        "opcode": "NoOp",
                        "outs": [],
                        "sync_info": {
                            "on_update": [],
                            "on_wait": extra[i : i + _MAX_WAITS],
                        },
                    }
                )
            si["on_wait"] = keep
        out.append(inst)
    insts[:] = out


def _walk_bir(obj):
    if isinstance(obj, dict):
        for key, val in obj.items():
            if key == "instructions" and isinstance(val, list):
                _fix_instructions(val)
            else:
                _walk_bir(val)
    elif isinstance(obj, list):
        for v in obj:
            _walk_bir(v)


def _install_patches():
    if getattr(bass.Bass, "_augru_patched", False):
        return
    orig_to_json = bass.Bass.to_json_bytes

    def to_json_bytes_fixed(self):
        bir = json.loads(orig_to_json(self))
        _walk_bir(bir)
        return json.dumps(bir).encode()

    bass.Bass.to_json_bytes = to_json_bytes_fixed

    def drain_and_barrier(self, tick_clock, wait_clock):
        drain_inst = self.nc.sync.drain()
        wait_clock.add_sem_waits(
            drain_inst.ins, ScopedClock({None: tick_clock.global_clock})
        )
        si = drain_inst.ins.sync_info
        waits = list(si.on_wait) if si is not None else []
        if len(waits) > 1:
            drain_inst.ins.sync_info = mybir.SyncInfo(
                on_wait=waits[:1], on_update=list(si.on_update)
            )
            for i in range(1, len(waits)):
                extra = self.nc.sync.drain()
                extra.ins.sync_info = mybir.SyncInfo(
                    on_wait=waits[i : i + 1], on_update=[]
                )
        self.nc.all_engine_barrier()
        assert self.sems is not None
        popped = self.nc._tile_sem_poison_stack.pop()
        assert popped is self._sem_poison
        self.nc.clear_and_free_semaphores(list(self.sems.allocated().values()))
        self.nc.all_engine_barrier()

    tile.TileContext._drain_and_barrier = drain_and_barrier
    bass.Bass._augru_patched = True


# ---------------------------------------------------------------------------

B, S, D = 1024, 200, 256
NCORES = 8
BL = B // NCORES  # 128 batch rows per core
DH = D // 128  # 2 chunks of 128 along the feature axis

F32 = mybir.dt.float32
F16 = mybir.dt.float16
AX = mybir.AxisListType
OP = mybir.AluOpType
AF = mybir.ActivationFunctionType

_N_STEPS = int(os.environ.get("BASS_AUGRU_STEPS", S))


def _bcast_rows(ap, n):
    """AP reading `ap`'s single leading row n times (partition broadcast)."""
    return bass.AP(tensor=ap.tensor, offset=ap.offset, ap=[[0, n]] + list(ap.ap)[1:])


def _bcast_mid(ap, n):
    """[P, F] -> [P, n, F] AP with a stride-0 middle dim."""
    a = list(ap.ap)
    return bass.AP(tensor=ap.tensor, offset=ap.offset, ap=[a[0], [0, n], a[1]])


def _build_program(n_steps):
    nc = bass.Bass(
        "TRN2", target_bir_lowering=False, debug=False, num_devices=NCORES
    )

    x32 = nc.dram_tensor("x32", [BL, S, D], F32, kind="ExternalInput").ap()
    x16 = nc.dram_tensor("x16", [BL, S, D], F16, kind="ExternalInput").ap()
    item = nc.dram_tensor("item", [BL, D], F32, kind="ExternalInput").ap()
    wa = nc.dram_tensor("wa", [D, D], F32, kind="ExternalInput").ap()
    # fp16 weights, one [d, e] matrix per gate input/recurrent pair
    wts = {
        name: nc.dram_tensor(name, [D, D], F16, kind="ExternalInput").ap()
        for name in ("wu", "uu", "wr", "ur", "wh", "uh")
    }
    # biases reshaped [DH, 128] so row k is e-chunk k
    bias_in = {
        name: nc.dram_tensor(name, [DH, 128], F16, kind="ExternalInput").ap()
        for name in ("bu", "br", "bh")
    }
    sel_in = nc.dram_tensor("sel", [DH, D], F16, kind="ExternalInput").ap()
    outs16 = nc.dram_tensor("outs16", [BL, S, D], F16, kind="ExternalOutput").ap()
    hlast16 = nc.dram_tensor("hlast16", [BL, D], F16, kind="ExternalOutput").ap()

    with tile.TileContext(nc) as tc, ExitStack() as ctx:
        consts = ctx.enter_context(tc.tile_pool(name="consts", bufs=1))
        dram = ctx.enter_context(tc.tile_pool(name="dram", bufs=1, space="DRAM"))
        att_ctx = ExitStack()
        att_ps = att_ctx.enter_context(tc.tile_pool(name="att_ps", bufs=2, space="PSUM"))
        xin = att_ctx.enter_context(tc.tile_pool(name="xin", bufs=8))
        att_tmp = att_ctx.enter_context(tc.tile_pool(name="att_tmp", bufs=2))

        ident = consts.tile([128, 128], F32)
        make_identity(nc, ident[:])

        # ---------------- attention phase (fp32) ----------------
        item_sb = consts.tile([128, D], F32)
        nc.sync.dma_start(item_sb[:], item)
        wa_sb = [consts.tile([128, D], F32, name=f"wa_sb{_k}") for _k in range(DH)]
        for k in range(DH):
            nc.sync.dma_start(wa_sb[k][:], wa[ts(k, 128), :])

        # itemT chunks: [e-half, b]
        item_t = consts.tile([128, DH, 128], F32)
        for k in range(DH):
            p = att_ps.tile([128, 128], F32)
            nc.tensor.transpose(p[:], item_sb[:, ts(k, 128)], ident[:])
            nc.scalar.copy(item_t[:, k, :], p[:])
        # WaT[j][:, d]: WaT[j][e', d] = Wa[d, 128j + e']
        wa_t = [consts.tile([128, D], F32, name=f"wa_t{_k}") for _k in range(DH)]
        for j in range(DH):
            for k in range(DH):
                p = att_ps.tile([128, 128], F32)
                nc.tensor.transpose(p[:], wa_sb[k][:, ts(j, 128)], ident[:])
                nc.scalar.copy(wa_t[j][:, ts(k, 128)], p[:])

        # v[b, d] = sum_e item[b, e] Wa[d, e]
        v_ps = att_ps.tile([128, D], F32)
        for j in range(DH):
            nc.tensor.matmul(
                v_ps[:], item_t[:, j, :], wa_t[j][:], start=(j == 0), stop=(j == DH - 1),
                skip_group_check=True,
            )
        v_sb = consts.tile([128, D], F32)
        nc.scalar.copy(v_sb[:], v_ps[:])

        # scores[b, t] = sum_d x[b, t, d] v[b, d]
        scores = consts.tile([128, S], F32)
        for t in range(S):
            xt = xin.tile([128, D], F32, tag="x32t")
            nc.sync.dma_start(xt[:], x32[:, t, :])
            scratch = att_tmp.tile([128, D], F32, tag="ttr_scratch")
            nc.vector.tensor_tensor(scratch[:], xt[:], v_sb[:], OP.mult)
            nc.vector.tensor_reduce(
                scores[:, t : t + 1], scratch[:], axis=AX.X, op=OP.add
            )

        # softmax over t (mask is all-True in this problem)
        mx = consts.tile([128, 1], F32)
        nc.vector.tensor_reduce(mx[:], scores[:], axis=AX.X, op=OP.max)
        negmax = consts.tile([128, 1], F32)
        nc.vector.tensor_scalar(negmax[:], mx[:], -1.0, None, OP.mult)
        exps = consts.tile([128, S], F32)
        sumexp = consts.tile([128, 1], F32)
        nc.scalar.activation(
            exps[:], scores[:], AF.Exp, bias=negmax[:], scale=1.0, accum_out=sumexp[:]
        )
        rsum = consts.tile([128, 1], F32)
        nc.vector.reciprocal(rsum[:], sumexp[:])
        attn = consts.tile([128, S], F32)
        nc.vector.tensor_scalar(attn[:], exps[:], rsum[:], None, OP.mult)

        # attn^T rows to DRAM ([t, b] fp16) for per-step partition-broadcast
        attn_dram = dram.tile([S, 128], F16)
        pa = att_ps.tile([128, 128], F32)
        nc.tensor.transpose(pa[:], attn[:, 0:128], ident[:])
        at0 = att_tmp.tile([128, 128], F16, tag="attnT")
        nc.scalar.copy(at0[:], pa[:])
        nc.sync.dma_start(attn_dram[0:128, :], at0[:])
        pa2 = att_ps.tile([128, 128], F32)
        nc.tensor.transpose(pa2[0 : S - 128, :], attn[:, 128:S], ident[:])
        at1 = att_tmp.tile([128, 128], F16, tag="attnT")
        nc.scalar.copy(at1[0 : S - 128, :], pa2[0 : S - 128, :])
        nc.sync.dma_start(attn_dram[128:S, :], at1[0 : S - 128, :])

        # release attention-phase PSUM/SBUF pools before the recurrence pools
        att_ctx.close()

        # ---------------- recurrence phase (fp16 matmuls) ----------------
        wpool = ctx.enter_context(tc.tile_pool(name="wpool", bufs=1))
        # weight tiles: w[name][k] is [128 (d-half k), D (e)] fp16
        wsb = {}
        for name in ("wu", "uu", "wr", "ur", "wh", "uh"):
            wsb[name] = [wpool.tile([128, D], F16, tag=f"{name}{k}", name=f"{name}{k}") for k in range(DH)]
            for k in range(DH):
                nc.sync.dma_start(wsb[name][k][:], wts[name][ts(k, 128), :])
        bias_sb = {}
        for name in ("bu", "br", "bh"):
            bias_sb[name] = wpool.tile([DH, 128], F16, tag=name, name=f"b_{name}")
            nc.sync.dma_start(bias_sb[name][:], bias_in[name])
        # selector [DH, D]: row k is 1.0 on e-chunk k, else 0 (host-provided)
        sel = wpool.tile([DH, D], F16)
        nc.sync.dma_start(sel[:], sel_in)

        ps = ctx.enter_context(tc.tile_pool(name="ps", bufs=2, space="PSUM"))
        xtp = ctx.enter_context(tc.tile_pool(name="xtp", bufs=6))
        abc = ctx.enter_context(tc.tile_pool(name="abc", bufs=4))
        ch = ctx.enter_context(tc.tile_pool(name="ch", bufs=2))
        hp = ctx.enter_context(tc.tile_pool(name="hp", bufs=2))
        hout = ctx.enter_context(tc.tile_pool(name="hout", bufs=3))

        h_cur = hp.tile([128, D], F16, tag="h")
        nc.vector.memset(h_cur[:], 0.0)

        def xpath(t):
            """fp16 x_t transposed chunks + attention row broadcast, for step t."""
            xt = xtp.tile([128, D], F16, tag="xT")
            for k in range(DH):
                nc.sync.dma_start_transpose(xt[:, ts(k, 128)], x16[:, t, ts(k, 128)])
            ab = abc.tile([128, 128], F16, tag="abc")
            nc.sync.dma_start(ab[:], _bcast_rows(attn_dram[t : t + 1, :], 128))
            return xt, ab

        # channel -> (x-weight, u-weight, bias)
        CH = {"r": ("wr", "ur", "br"), "u": ("wu", "uu", "bu"), "h": ("wh", "uh", "bh")}

        def xw_mms(t, xt):
            """bias + x-side matmuls for step t -> 4 psum tiles."""
            P = {
                "r": ps.tile([128, D], F32, tag="Pr", name="Pr"),
                "u": ps.tile([128, D], F32, tag="Pu", name="Pu"),
                "h1": ps.tile([128, D], F32, tag="Ph1", name="Ph1"),
                "h2": ps.tile([128, D], F32, tag="Ph2", name="Ph2"),
            }
            for c, (wn, un, bn) in CH.items():
                tgt = P["h1"] if c == "h" else P[c]
                nc.tensor.matmul(
                    tgt[:], bias_sb[bn][:], sel[:], start=True, stop=False,
                    skip_group_check=True,
                )
                for j in range(DH):
                    for k in range(DH):
                        nc.tensor.matmul(
                            tgt[:, ts(j, 128)],
                            wsb[wn][k][:, ts(j, 128)],
                            xt[:, ts(k, 128)],
                            start=False,
                            stop=(c == "h" and k == DH - 1),
                            skip_group_check=True,
                        )
            return P

        def u_mms(t, P, h_prev):
            """h-side matmuls for step t (the serial part)."""
            for c in ("r", "u", "h"):
                un = CH[c][1]
                tgt = P["h2"] if c == "h" else P[c]
                for j in range(DH):
                    for k in range(DH):
                        nc.tensor.matmul(
                            tgt[:, ts(j, 128)],
                            wsb[un][k][:, ts(j, 128)],
                            h_prev[:, ts(k, 128)],
                            start=(c == "h" and k == 0),
                            stop=(k == DH - 1),
                            skip_group_check=True,
                        )

        def chain(t, P, h_prev, ab):
            r = ch.tile([128, D], F16, tag="r")
            nc.scalar.activation(r[:], P["r"][:], AF.Sigmoid)
            u = ch.tile([128, D], F16, tag="u")
            nc.scalar.activation(u[:], P["u"][:], AF.Sigmoid)
            t2 = ch.tile([128, D], F16, tag="t2")
            nc.vector.tensor_tensor(t2[:], r[:], P["h2"][:], OP.mult)
            pre = ch.tile([128, D], F16, tag="pre")
            nc.vector.tensor_tensor(pre[:], t2[:], P["h1"][:], OP.add)
            hh = ch.tile([128, D], F16, tag="hh")
            nc.scalar.activation(hh[:], pre[:], AF.Tanh)
            uh = ch.tile([128, D], F16, tag="uh")
            nc.gpsimd.tensor_tensor(
                uh[:].rearrange("p (c b) -> p c b", c=DH),
                u[:].rearrange("p (c b) -> p c b", c=DH),
                _bcast_mid(ab[:], DH),
                OP.mult,
            )
            s = ch.tile([128, D], F16, tag="s")
            nc.gpsimd.tensor_scalar(s[:], uh[:], -1.0, 1.0, OP.mult, OP.add)
            m1 = ch.tile([128, D], F16, tag="m1")
            nc.gpsimd.tensor_tensor(m1[:], h_prev[:], s[:], OP.mult)
            m2 = ch.tile([128, D], F16, tag="m2")
            nc.vector.tensor_tensor(m2[:], uh[:], hh[:], OP.mult)
            h_new = hp.tile([128, D], F16, tag="h")
            nc.vector.tensor_tensor(h_new[:], m1[:], m2[:], OP.add)
            return h_new

        def h_out(t, h_new):
            hbt = hout.tile([128, D], F16, tag="hbt")
            for k in range(DH):
                nc.sync.dma_start_transpose(hbt[:, ts(k, 128)], h_new[:, ts(k, 128)])
            nc.sync.dma_start(outs16[:, t, :], hbt[:])
            if t == n_steps - 1:
                nc.sync.dma_start(hlast16, hbt[:])

        xt0, ab0 = xpath(0)
        P_cur = xw_mms(0, xt0)
        ab_cur = ab0
        for t in range(n_steps):
            u_mms(t, P_cur, h_cur)
            if t + 1 < n_steps:
                xt_n, ab_n = xpath(t + 1)
                P_next = xw_mms(t + 1, xt_n)
            h_new = chain(t, P_cur, h_cur, ab_cur)
            h_out(t, h_new)
            h_cur = h_new
            if t + 1 < n_steps:
                P_cur, ab_cur = P_next, ab_n

    return nc


_CACHE = {}


def _get_program(n_steps):
    if n_steps not in _CACHE:
        _install_patches()
        _CACHE[n_steps] = _build_program(n_steps)
    return _CACHE[n_steps]


def _numpy_fallback(x, item, mask, Wa, Wu, Uu, bu, Wr, Ur, br, Wh, Uh, bh):
    """Plain numpy AUGRU for inputs the device kernel doesn't handle
    (only used if mask is not all-True)."""
    x = np.asarray(x, np.float32)
    scores = np.einsum("bsd,de->bse", x, Wa)
    scores = (scores * np.asarray(item)[:, None, :]).sum(-1)
    scores = np.where(np.asarray(mask), scores, -np.inf)
    m = scores.max(1, keepdims=True)
    e = np.exp(scores - m)
    a = e / e.sum(1, keepdims=True)
    nan_rows = ~np.isfinite(a).all(1, keepdims=True)
    a = np.where(nan_rows, 1.0 / scores.shape[1], a)
    xu = np.einsum("bsd,de->bse", x, Wu) + bu
    xr = np.einsum("bsd,de->bse", x, Wr) + br
    xh = np.einsum("bsd,de->bse", x, Wh) + bh
    h = np.zeros((x.shape[0], x.shape[2]), np.float32)
    outs = np.zeros_like(xu)
    for t in range(x.shape[1]):
        u = 1.0 / (1.0 + np.exp(-(xu[:, t] + h @ Uu)))
        r = 1.0 / (1.0 + np.exp(-(xr[:, t] + h @ Ur)))
        hh = np.tanh(xh[:, t] + r * (h @ Uh))
        uhat = a[:, t : t + 1] * u
        h = (1.0 - uhat) * h + uhat * hh
        outs[:, t] = h
    return outs, h


def kernel(x, item, mask, Wa, Wu, Uu, bu, Wr, Ur, br, Wh, Uh, bh):
    import concourse.bass_utils as bass_utils

    x = np.ascontiguousarray(np.asarray(x, dtype=np.float32))
    item = np.ascontiguousarray(np.asarray(item, dtype=np.float32))
    mask = np.asarray(mask)
    if not bool(mask.all()):
        return _numpy_fallback(
            x, item, mask, np.asarray(Wa), np.asarray(Wu), np.asarray(Uu),
            np.asarray(bu), np.asarray(Wr), np.asarray(Ur), np.asarray(br),
            np.asarray(Wh), np.asarray(Uh), np.asarray(bh),
        )

    n_steps = _N_STEPS
    nc = _get_program(n_steps)

    x16 = x.astype(np.float16)
    com = {
        "wa": np.ascontiguousarray(np.asarray(Wa, np.float32)),
        "wu": np.asarray(Wu, np.float32).astype(np.float16),
        "uu": np.asarray(Uu, np.float32).astype(np.float16),
        "wr": np.asarray(Wr, np.float32).astype(np.float16),
        "ur": np.asarray(Ur, np.float32).astype(np.float16),
        "wh": np.asarray(Wh, np.float32).astype(np.float16),
        "uh": np.asarray(Uh, np.float32).astype(np.float16),
        "bu": np.asarray(bu, np.float32).astype(np.float16).reshape(DH, 128),
        "br": np.asarray(br, np.float32).astype(np.float16).reshape(DH, 128),
        "bh": np.asarray(bh, np.float32).astype(np.float16).reshape(DH, 128),
        "sel": np.kron(np.eye(DH, dtype=np.float16), np.ones((1, 128), np.float16)),
    }
    com = {k: np.ascontiguousarray(v) for k, v in com.items()}
    in_maps = []
    for c in range(NCORES):
        sl = slice(c * BL, (c + 1) * BL)
        in_maps.append(
            {
                "x32": x[sl],
                "x16": np.ascontiguousarray(x16[sl]),
                "item": item[sl],
                **com,
            }
        )

    trace = bool(int(os.environ.get("BASS_AUGRU_TRACE", "0")))
    res = bass_utils.run_bass_kernel_spmd(
        nc, in_maps, core_ids=list(range(NCORES)), trace=trace
    )
    kernel.last_results = res

    outs = np.empty((B, S, D), np.float32)
    hlast = np.empty((B, D), np.float32)
    for c in range(NCORES):
        sl = slice(c * BL, (c + 1) * BL)
        outs[sl] = res.results[c]["outs16"].astype(np.float32)
        hlast[sl] = res.results[c]["hlast16"].astype(np.float32)
    return outs, hlast
